# revision 5
# baseline (speedup 1.0000x reference)
# Trainium2 Bass kernel for single-query dot-product attention (decode step).
#
#   attn = softmax(q @ C^T)          q: (B, 1, H)  C: (B, S, H)
#   out  = tanh(attn @ C)
#   returns (out, attn)              B=32, S=4096, H=1024, fp32
#
# Sharding: batch-parallel, 4 batches per core across 8 NeuronCores.
#
# Per-core dataflow (per batch, single HBM pass over C):
#   - DMA C in [128, 2, 1024] chunks (partition = s mod 128, 1 MiB per dma)
#   - scores: DVE tensor_tensor_reduce computes prod_t = C_t * q_rep
#     (written as float32r for the later PE matmul) and accumulates
#     scores[:, t] = sum_h prod in the same pass
#   - softmax: DVE free-axis max, GPSIMD partition all-reduce (max),
#     ACT exp-with-accum (row sums), GPSIMD all-reduce (add), normalize
#   - out' = P^T @ prod on PE in fp32r (attn weights stationary, prod
#     streaming, PSUM accumulation over all 32 s-subtiles). Since
#     prod = C .* q, out' = out .* q; divide by q (DVE, 2-ULP approx
#     reciprocal) and tanh on ACT.
#   - attn transposed on PE so the HBM writeout is contiguous.
from contextlib import ExitStack

import numpy as np

import concourse.bass as bass
import concourse.bacc as bacc
import concourse.mybir as mybir
import concourse.bass_isa as bass_isa
import concourse.tile as tile
from concourse.bass_utils import run_bass_kernel_spmd
from concourse.masks import make_identity

B, S, H = 32, 4096, 1024
NCORES = 8
BPC = B // NCORES          # batches per core
NT = S // 128              # 32 s-subtiles of 128 rows per batch
TPD = 2                    # s-subtiles per DMA chunk (1 MiB per dma_start)
ND = NT // TPD             # dma chunks per batch

F32 = mybir.dt.float32
F32R = mybir.dt.float32r

LAST_RESULTS = None        # test.py reads profiling info from here


def _build(tc, q_ap, c_ap, out_ap, attn_ap, repeat=1):
    nc = tc.nc
    ctx = ExitStack()
    with ctx:
        cpool = ctx.enter_context(tc.tile_pool(name="cpool", bufs=3))
        prodp = ctx.enter_context(tc.tile_pool(name="prodp", bufs=32))
        qpool = ctx.enter_context(tc.tile_pool(name="qpool", bufs=2))
        smallp = ctx.enter_context(tc.tile_pool(name="smallp", bufs=2))
        singles = ctx.enter_context(tc.tile_pool(name="singles", bufs=1))
        psump = ctx.enter_context(tc.tile_pool(name="psump", bufs=2, space="PSUM"))

        identity = singles.tile([128, 128], F32)
        make_identity(nc, identity)

        if repeat > 1:
            # timing amplification only: run the whole per-core program
            # `repeat` times inside a device-side loop
            loop = ctx.enter_context(tc.For_i(0, repeat, 1))

        for b in range(BPC):
            # q broadcast to all 128 partitions: [128, H]
            q_rep = qpool.tile([128, H], F32)
            q_src = bass.AP(
                tensor=q_ap.tensor, offset=b * H, ap=[[0, 128], [1, H]]
            )
            nc.gpsimd.dma_start(out=q_rep, in_=q_src)

            # 1/q for the final un-scaling (prod tiles carry a factor of q)
            q_inv = smallp.tile([1, H], F32, tag="q_inv")
            q_inv_scratch = smallp.tile([1, H], F32, tag="q_inv_scratch")
            nc.vector.reciprocal_approx_accurate(
                out=q_inv, in_=q_rep[0:1, :], scratch=q_inv_scratch
            )

            # C for this batch: s = t*128 + p  ->  [p, t, h]
            c_resh = c_ap[b].rearrange("(t p) h -> p t h", p=128)

            scores = smallp.tile([128, NT], F32, tag="scores")
            prods = []
            for j in range(ND):
                c_tile = cpool.tile([128, TPD, H], F32, tag="C")
                nc.sync.dma_start(
                    out=c_tile, in_=c_resh[:, j * TPD : (j + 1) * TPD, :]
                )
                for k in range(TPD):
                    t = j * TPD + k
                    prod = prodp.tile([128, H], F32R, tag="prod")
                    # prod = C_t * q (rounded to fp32r for the PE matmul),
                    # scores[:, t] = sum_h prod  — one DVE pass
                    nc.vector.scalar_tensor_tensor(
                        out=prod,
                        in0=c_tile[:, k, :],
                        scalar=0.0,
                        in1=q_rep,
                        op0=mybir.AluOpType.bypass,
                        op1=mybir.AluOpType.mult,
                        accum_out=scores[:, t : t + 1],
                    )
                    prods.append(prod)

            # global max over the whole [128, NT] score block
            mx = smallp.tile([128, 1], F32, tag="mx")
            nc.vector.reduce_max(mx, scores, axis=mybir.AxisListType.X)
            m_all = smallp.tile([128, 1], F32, tag="m_all")
            nc.gpsimd.partition_all_reduce(
                m_all, mx, channels=128, reduce_op=bass_isa.ReduceOp.max
            )
            negm = smallp.tile([128, 1], F32, tag="negm")
            nc.vector.tensor_scalar_mul(negm, m_all, -1.0)

            # E = exp(scores - max), z_col[p] = sum_t E[p, t]
            e_blk = smallp.tile([128, NT], F32, tag="e_blk")
            z_col = smallp.tile([128, 1], F32, tag="z_col")
            nc.scalar.activation(
                out=e_blk,
                in_=scores,
                func=mybir.ActivationFunctionType.Exp,
                bias=negm,
                scale=1.0,
                accum_out=z_col,
            )
            z_all = smallp.tile([128, 1], F32, tag="z_all")
            nc.gpsimd.partition_all_reduce(
                z_all, z_col, channels=128, reduce_op=bass_isa.ReduceOp.add
            )
            r_all = smallp.tile([128, 1], F32, tag="r_all")
            nc.vector.reciprocal(r_all, z_all)

            # normalized attention weights: fp32 copy for the writeout,
            # fp32r copy as matmul weights
            p_attn = smallp.tile([128, NT], F32, tag="p_attn")
            nc.vector.tensor_scalar_mul(p_attn, e_blk, r_all)
            p_r = smallp.tile([128, NT], F32R, tag="p_r")
            nc.vector.tensor_scalar_mul(p_r, e_blk, r_all)

            # attn writeout: transpose [128, NT] -> [NT, 128] so HBM rows are
            # contiguous per partition
            pt_psum = psump.tile([NT, 128], F32, tag="pt")
            nc.tensor.transpose(pt_psum, p_attn, identity)
            attn_sb = smallp.tile([NT, 128], F32, tag="attn_sb")
            nc.scalar.copy(attn_sb, pt_psum)
            nc.sync.dma_start(
                out=attn_ap[b].rearrange("q (t j) -> (q t) j", j=128), in_=attn_sb
            )

            # out' = P^T @ prod accumulated over all 32 s-subtiles, fp32r.
            # t outer so prod tiles free in order for the next batch.
            u_psum = psump.tile([1, H], F32, tag="U")
            for t in range(NT):
                for n in range(H // 512):
                    nc.tensor.matmul(
                        u_psum[:, n * 512 : (n + 1) * 512],
                        lhsT=p_r[:, t : t + 1],
                        rhs=prods[t][:, n * 512 : (n + 1) * 512],
                        start=(t == 0),
                        stop=(t == NT - 1),
                    )

            # out = tanh(out' / q)
            u_sb = smallp.tile([1, H], F32, tag="u_sb")
            nc.vector.tensor_tensor(
                out=u_sb, in0=u_psum, in1=q_inv, op=mybir.AluOpType.mult
            )
            out_sb = smallp.tile([1, H], F32, tag="out_sb")
            nc.scalar.activation(
                out=out_sb, in_=u_sb, func=mybir.ActivationFunctionType.Tanh
            )
            nc.sync.dma_start(out=out_ap[b], in_=out_sb)


def kernel(
    output: np.ndarray,
    context: np.ndarray,
    _trace: bool = False,
    _repeat: int = 1,
):
    global LAST_RESULTS
    output = np.ascontiguousarray(np.asarray(output, dtype=np.float32))
    context = np.ascontiguousarray(np.asarray(context, dtype=np.float32))
    assert output.shape == (B, 1, H) and context.shape == (B, S, H)

    nc = bacc.Bacc(
        "TRN2",
        target_bir_lowering=False,
        debug=False,
        enable_asserts=False,
        num_devices=NCORES,
    )
    q_t = nc.dram_tensor("q", [BPC, 1, H], F32, kind="ExternalInput")
    c_t = nc.dram_tensor("c", [BPC, S, H], F32, kind="ExternalInput")
    out_t = nc.dram_tensor("out", [BPC, 1, H], F32, kind="ExternalOutput")
    attn_t = nc.dram_tensor("attn", [BPC, 1, S], F32, kind="ExternalOutput")

    with tile.TileContext(nc) as tc:
        _build(tc, q_t.ap(), c_t.ap(), out_t.ap(), attn_t.ap(), repeat=_repeat)
    nc.compile()

    in_maps = [
        {
            "q": output[i * BPC : (i + 1) * BPC],
            "c": context[i * BPC : (i + 1) * BPC],
        }
        for i in range(NCORES)
    ]
    res = run_bass_kernel_spmd(
        nc, in_maps, core_ids=list(range(NCORES)), trace=_trace
    )
    LAST_RESULTS = res
    out = np.concatenate([r["out"] for r in res.results], axis=0)
    attn = np.concatenate([r["attn"] for r in res.results], axis=0)
    return out, attn


if __name__ == "__main__":
    rng = np.random.default_rng(0)
    q = rng.standard_normal((B, 1, H), dtype=np.float32)
    c = rng.standard_normal((B, S, H), dtype=np.float32)
    o, a = kernel(q, c)
    print(o.shape, a.shape, float(np.abs(o).max()), float(a.sum(axis=-1).mean()))


# revision 9
# speedup vs baseline: 1.1279x; 1.1279x over previous
# Trainium2 Bass kernel for single-query dot-product attention (decode step).
#
#   attn = softmax(q @ C^T)          q: (B, 1, H)  C: (B, S, H)
#   out  = tanh(attn @ C)
#   returns (out, attn)              B=32, S=4096, H=1024, fp32
#
# Sharding: batch-parallel, 4 batches per core across 8 NeuronCores.
#
# Per-core dataflow (per batch, single HBM pass over C):
#   - DMA C in [128, 2, 1024] chunks (partition = s mod 128, 1 MiB per dma)
#   - scores: DVE tensor_tensor_reduce computes prod_t = C_t * q_rep
#     (written as float32r for the later PE matmul) and accumulates
#     scores[:, t] = sum_h prod in the same pass
#   - softmax: DVE free-axis max, GPSIMD partition all-reduce (max),
#     ACT exp-with-accum (row sums), GPSIMD all-reduce (add), normalize
#   - out' = P^T @ prod on PE in fp32r (attn weights stationary, prod
#     streaming, PSUM accumulation over all 32 s-subtiles). Since
#     prod = C .* q, out' = out .* q; divide by q (DVE, 2-ULP approx
#     reciprocal) and tanh on ACT.
#   - attn transposed on PE so the HBM writeout is contiguous.
from contextlib import ExitStack

import numpy as np

import concourse.bass as bass
import concourse.bacc as bacc
import concourse.mybir as mybir
import concourse.bass_isa as bass_isa
import concourse.tile as tile
from concourse.bass_utils import run_bass_kernel_spmd
from concourse.masks import make_identity

B, S, H = 32, 4096, 1024
NCORES = 8
BPC = B // NCORES          # batches per core
NT = S // 128              # 32 s-subtiles of 128 rows per batch
TPD = 2                    # s-subtiles per DMA chunk (1 MiB per dma_start)
ND = NT // TPD             # dma chunks per batch

F32 = mybir.dt.float32
F32R = mybir.dt.float32r

CPOOL_BUFS = 4
PROD_BUFS = 34

LAST_RESULTS = None        # test.py reads profiling info from here


def _build(tc, q_ap, c_ap, out_ap, attn_ap, repeat=1):
    nc = tc.nc
    ctx = ExitStack()
    with ctx:
        cpool = ctx.enter_context(tc.tile_pool(name="cpool", bufs=CPOOL_BUFS))
        prodp = ctx.enter_context(tc.tile_pool(name="prodp", bufs=PROD_BUFS))
        qpool = ctx.enter_context(tc.tile_pool(name="qpool", bufs=2))
        smallp = ctx.enter_context(tc.tile_pool(name="smallp", bufs=2))
        singles = ctx.enter_context(tc.tile_pool(name="singles", bufs=1))
        psump = ctx.enter_context(tc.tile_pool(name="psump", bufs=2, space="PSUM"))

        identity = singles.tile([128, 128], F32)
        make_identity(nc, identity)

        if repeat > 1:
            # timing amplification only: run the whole per-core program
            # `repeat` times inside a device-side loop
            loop = ctx.enter_context(tc.For_i(0, repeat, 1))

        for b in range(BPC):
            # q broadcast to all 128 partitions: [128, H]
            q_rep = qpool.tile([128, H], F32)
            q_src = bass.AP(
                tensor=q_ap.tensor, offset=b * H, ap=[[0, 128], [1, H]]
            )
            nc.gpsimd.dma_start(out=q_rep, in_=q_src)

            # 1/q for the final un-scaling (prod tiles carry a factor of q)
            q_inv = smallp.tile([1, H], F32, tag="q_inv")
            q_inv_scratch = smallp.tile([1, H], F32, tag="q_inv_scratch")
            nc.vector.reciprocal_approx_accurate(
                out=q_inv, in_=q_rep[0:1, :], scratch=q_inv_scratch
            )

            # C for this batch: s = t*128 + p  ->  [p, t, h]
            c_resh = c_ap[b].rearrange("(t p) h -> p t h", p=128)

            scores = smallp.tile([128, NT], F32, tag="scores")
            prods = []
            for j in range(ND):
                c_tile = cpool.tile([128, TPD, H], F32, tag="C")
                nc.sync.dma_start(
                    out=c_tile, in_=c_resh[:, j * TPD : (j + 1) * TPD, :]
                )
                for k in range(TPD):
                    t = j * TPD + k
                    prod = prodp.tile([128, H], F32R, tag="prod")
                    # prod = C_t * q (rounded to fp32r for the PE matmul),
                    # scores[:, t] = sum_h prod  — one DVE pass
                    nc.vector.scalar_tensor_tensor(
                        out=prod,
                        in0=c_tile[:, k, :],
                        scalar=0.0,
                        in1=q_rep,
                        op0=mybir.AluOpType.bypass,
                        op1=mybir.AluOpType.mult,
                        accum_out=scores[:, t : t + 1],
                    )
                    prods.append(prod)

            # global max over the whole [128, NT] score block
            mx = smallp.tile([128, 1], F32, tag="mx")
            nc.vector.reduce_max(mx, scores, axis=mybir.AxisListType.X)
            m_all = smallp.tile([128, 1], F32, tag="m_all")
            nc.gpsimd.partition_all_reduce(
                m_all, mx, channels=128, reduce_op=bass_isa.ReduceOp.max
            )
            negm = smallp.tile([128, 1], F32, tag="negm")
            nc.vector.tensor_scalar_mul(negm, m_all, -1.0)

            # E = exp(scores - max), z_col[p] = sum_t E[p, t]
            e_blk = smallp.tile([128, NT], F32, tag="e_blk")
            z_col = smallp.tile([128, 1], F32, tag="z_col")
            nc.scalar.activation(
                out=e_blk,
                in_=scores,
                func=mybir.ActivationFunctionType.Exp,
                bias=negm,
                scale=1.0,
                accum_out=z_col,
            )
            # fp32r copy of E: the output matmul runs on UNNORMALIZED
            # weights so it can start right after the exp; 1/Z is folded
            # into the final [1, H] scale instead.
            e_r = smallp.tile([128, NT], F32R, tag="e_r")
            nc.vector.tensor_scalar_mul(e_r, e_blk, 1.0)

            # out'' = E^T @ prod accumulated over all 32 s-subtiles, fp32r.
            # t outer so prod tiles free in order for the next batch.
            u_psum = psump.tile([1, H], F32, tag="U")
            for t in range(NT):
                for n in range(H // 512):
                    nc.tensor.matmul(
                        u_psum[:, n * 512 : (n + 1) * 512],
                        lhsT=e_r[:, t : t + 1],
                        rhs=prods[t][:, n * 512 : (n + 1) * 512],
                        start=(t == 0),
                        stop=(t == NT - 1),
                    )

            # Z and attention normalization (off the PE critical path)
            z_all = smallp.tile([128, 1], F32, tag="z_all")
            nc.gpsimd.partition_all_reduce(
                z_all, z_col, channels=128, reduce_op=bass_isa.ReduceOp.add
            )
            r_all = smallp.tile([128, 1], F32, tag="r_all")
            nc.vector.reciprocal(r_all, z_all)
            p_attn = smallp.tile([128, NT], F32, tag="p_attn")
            nc.vector.tensor_scalar_mul(p_attn, e_blk, r_all)

            # attn writeout: transpose [128, NT] -> [NT, 128] so HBM rows are
            # contiguous per partition
            pt_psum = psump.tile([NT, 128], F32, tag="pt")
            nc.tensor.transpose(pt_psum, p_attn, identity)
            attn_sb = smallp.tile([NT, 128], F32, tag="attn_sb")
            nc.scalar.copy(attn_sb, pt_psum)
            nc.sync.dma_start(
                out=attn_ap[b].rearrange("q (t j) -> (q t) j", j=128), in_=attn_sb
            )

            # out = tanh(out'' * (1/Z) / q) — single DVE pass + ACT tanh
            u2_psum = psump.tile([1, H], F32, tag="U2", bufs=1)
            nc.vector.scalar_tensor_tensor(
                out=u2_psum,
                in0=u_psum,
                scalar=r_all[0:1, :],
                in1=q_inv,
                op0=mybir.AluOpType.mult,
                op1=mybir.AluOpType.mult,
            )
            out_sb = smallp.tile([1, H], F32, tag="out_sb")
            nc.scalar.activation(
                out=out_sb, in_=u2_psum, func=mybir.ActivationFunctionType.Tanh
            )
            nc.sync.dma_start(out=out_ap[b], in_=out_sb)


def kernel(
    output: np.ndarray,
    context: np.ndarray,
    _trace: bool = False,
    _repeat: int = 1,
):
    global LAST_RESULTS
    output = np.ascontiguousarray(np.asarray(output, dtype=np.float32))
    context = np.ascontiguousarray(np.asarray(context, dtype=np.float32))
    assert output.shape == (B, 1, H) and context.shape == (B, S, H)

    nc = bacc.Bacc(
        "TRN2",
        target_bir_lowering=False,
        debug=False,
        enable_asserts=False,
        num_devices=NCORES,
    )
    q_t = nc.dram_tensor("q", [BPC, 1, H], F32, kind="ExternalInput")
    c_t = nc.dram_tensor("c", [BPC, S, H], F32, kind="ExternalInput")
    out_t = nc.dram_tensor("out", [BPC, 1, H], F32, kind="ExternalOutput")
    attn_t = nc.dram_tensor("attn", [BPC, 1, S], F32, kind="ExternalOutput")

    with tile.TileContext(nc) as tc:
        _build(tc, q_t.ap(), c_t.ap(), out_t.ap(), attn_t.ap(), repeat=_repeat)
    nc.compile()

    in_maps = [
        {
            "q": output[i * BPC : (i + 1) * BPC],
            "c": context[i * BPC : (i + 1) * BPC],
        }
        for i in range(NCORES)
    ]
    res = run_bass_kernel_spmd(
        nc, in_maps, core_ids=list(range(NCORES)), trace=_trace
    )
    LAST_RESULTS = res
    out = np.concatenate([r["out"] for r in res.results], axis=0)
    attn = np.concatenate([r["attn"] for r in res.results], axis=0)
    return out, attn


if __name__ == "__main__":
    rng = np.random.default_rng(0)
    q = rng.standard_normal((B, 1, H), dtype=np.float32)
    c = rng.standard_normal((B, S, H), dtype=np.float32)
    o, a = kernel(q, c)
    print(o.shape, a.shape, float(np.abs(o).max()), float(a.sum(axis=-1).mean()))


# revision 10
# speedup vs baseline: 1.1686x; 1.0361x over previous
# Trainium2 Bass kernel for single-query dot-product attention (decode step).
#
#   attn = softmax(q @ C^T)          q: (B, 1, H)  C: (B, S, H)
#   out  = tanh(attn @ C)
#   returns (out, attn)              B=32, S=4096, H=1024, fp32
#
# Sharding: batch-parallel, 4 batches per core across 8 NeuronCores.
#
# Per-core dataflow (per batch, single HBM pass over C):
#   - DMA C in [128, 2, 1024] chunks (partition = s mod 128, 1 MiB per dma)
#   - scores: DVE tensor_tensor_reduce computes prod_t = C_t * q_rep
#     (written as float32r for the later PE matmul) and accumulates
#     scores[:, t] = sum_h prod in the same pass
#   - softmax: DVE free-axis max, GPSIMD partition all-reduce (max),
#     ACT exp-with-accum (row sums), GPSIMD all-reduce (add), normalize
#   - out' = P^T @ prod on PE in fp32r (attn weights stationary, prod
#     streaming, PSUM accumulation over all 32 s-subtiles). Since
#     prod = C .* q, out' = out .* q; divide by q (DVE, 2-ULP approx
#     reciprocal) and tanh on ACT.
#   - attn transposed on PE so the HBM writeout is contiguous.
from contextlib import ExitStack

import numpy as np

import concourse.bass as bass
import concourse.bacc as bacc
import concourse.mybir as mybir
import concourse.bass_isa as bass_isa
import concourse.tile as tile
from concourse.bass_utils import run_bass_kernel_spmd
from concourse.masks import make_identity

B, S, H = 32, 4096, 1024
NCORES = 8
BPC = B // NCORES          # batches per core
NT = S // 128              # 32 s-subtiles of 128 rows per batch
TPD = 2                    # s-subtiles per DMA chunk (1 MiB per dma_start)
ND = NT // TPD             # dma chunks per batch

F32 = mybir.dt.float32
F32R = mybir.dt.float32r

CPOOL_BUFS = 6
PROD_BUFS = 48
PROD_DT = mybir.dt.bfloat16  # fp32r for higher precision, bf16 for SBUF depth

LAST_RESULTS = None        # test.py reads profiling info from here


def _build(tc, q_ap, c_ap, out_ap, attn_ap, repeat=1):
    nc = tc.nc
    ctx = ExitStack()
    with ctx:
        cpool = ctx.enter_context(tc.tile_pool(name="cpool", bufs=CPOOL_BUFS))
        prodp = ctx.enter_context(tc.tile_pool(name="prodp", bufs=PROD_BUFS))
        qpool = ctx.enter_context(tc.tile_pool(name="qpool", bufs=2))
        smallp = ctx.enter_context(tc.tile_pool(name="smallp", bufs=2))
        singles = ctx.enter_context(tc.tile_pool(name="singles", bufs=1))
        psump = ctx.enter_context(tc.tile_pool(name="psump", bufs=2, space="PSUM"))

        identity = singles.tile([128, 128], F32)
        make_identity(nc, identity)

        if repeat > 1:
            # timing amplification only: run the whole per-core program
            # `repeat` times inside a device-side loop
            loop = ctx.enter_context(tc.For_i(0, repeat, 1))

        for b in range(BPC):
            # q broadcast to all 128 partitions: [128, H]
            q_rep = qpool.tile([128, H], F32)
            q_src = bass.AP(
                tensor=q_ap.tensor, offset=b * H, ap=[[0, 128], [1, H]]
            )
            nc.gpsimd.dma_start(out=q_rep, in_=q_src)

            # 1/q for the final un-scaling (prod tiles carry a factor of q)
            q_inv = smallp.tile([1, H], F32, tag="q_inv")
            q_inv_scratch = smallp.tile([1, H], F32, tag="q_inv_scratch")
            nc.vector.reciprocal_approx_accurate(
                out=q_inv, in_=q_rep[0:1, :], scratch=q_inv_scratch
            )

            # C for this batch: s = t*128 + p  ->  [p, t, h]
            c_resh = c_ap[b].rearrange("(t p) h -> p t h", p=128)

            scores = smallp.tile([128, NT], F32, tag="scores")
            prods = []
            for j in range(ND):
                c_tile = cpool.tile([128, TPD, H], F32, tag="C")
                nc.sync.dma_start(
                    out=c_tile, in_=c_resh[:, j * TPD : (j + 1) * TPD, :]
                )
                for k in range(TPD):
                    t = j * TPD + k
                    prod = prodp.tile([128, H], PROD_DT, tag="prod")
                    # prod = C_t * q (rounded to fp32r for the PE matmul),
                    # scores[:, t] = sum_h prod  — one DVE pass
                    nc.vector.scalar_tensor_tensor(
                        out=prod,
                        in0=c_tile[:, k, :],
                        scalar=0.0,
                        in1=q_rep,
                        op0=mybir.AluOpType.bypass,
                        op1=mybir.AluOpType.mult,
                        accum_out=scores[:, t : t + 1],
                    )
                    prods.append(prod)

            # global max over the whole [128, NT] score block
            mx = smallp.tile([128, 1], F32, tag="mx")
            nc.vector.reduce_max(mx, scores, axis=mybir.AxisListType.X)
            m_all = smallp.tile([128, 1], F32, tag="m_all")
            nc.gpsimd.partition_all_reduce(
                m_all, mx, channels=128, reduce_op=bass_isa.ReduceOp.max
            )
            negm = smallp.tile([128, 1], F32, tag="negm")
            nc.vector.tensor_scalar_mul(negm, m_all, -1.0)

            # E = exp(scores - max), z_col[p] = sum_t E[p, t]
            e_blk = smallp.tile([128, NT], F32, tag="e_blk")
            z_col = smallp.tile([128, 1], F32, tag="z_col")
            nc.scalar.activation(
                out=e_blk,
                in_=scores,
                func=mybir.ActivationFunctionType.Exp,
                bias=negm,
                scale=1.0,
                accum_out=z_col,
            )
            # fp32r copy of E: the output matmul runs on UNNORMALIZED
            # weights so it can start right after the exp; 1/Z is folded
            # into the final [1, H] scale instead.
            e_r = smallp.tile([128, NT], PROD_DT, tag="e_r")
            nc.vector.tensor_scalar_mul(e_r, e_blk, 1.0)

            # out'' = E^T @ prod accumulated over all 32 s-subtiles, fp32r.
            # t outer so prod tiles free in order for the next batch.
            u_psum = psump.tile([1, H], F32, tag="U")
            for t in range(NT):
                for n in range(H // 512):
                    nc.tensor.matmul(
                        u_psum[:, n * 512 : (n + 1) * 512],
                        lhsT=e_r[:, t : t + 1],
                        rhs=prods[t][:, n * 512 : (n + 1) * 512],
                        start=(t == 0),
                        stop=(t == NT - 1),
                    )

            # Z and attention normalization (off the PE critical path)
            z_all = smallp.tile([128, 1], F32, tag="z_all")
            nc.gpsimd.partition_all_reduce(
                z_all, z_col, channels=128, reduce_op=bass_isa.ReduceOp.add
            )
            r_all = smallp.tile([128, 1], F32, tag="r_all")
            nc.vector.reciprocal(r_all, z_all)
            p_attn = smallp.tile([128, NT], F32, tag="p_attn")
            nc.vector.tensor_scalar_mul(p_attn, e_blk, r_all)

            # attn writeout: transpose [128, NT] -> [NT, 128] so HBM rows are
            # contiguous per partition
            pt_psum = psump.tile([NT, 128], F32, tag="pt")
            nc.tensor.transpose(pt_psum, p_attn, identity)
            attn_sb = smallp.tile([NT, 128], F32, tag="attn_sb")
            nc.scalar.copy(attn_sb, pt_psum)
            nc.sync.dma_start(
                out=attn_ap[b].rearrange("q (t j) -> (q t) j", j=128), in_=attn_sb
            )

            # out = tanh(out'' * (1/Z) / q) — single DVE pass + ACT tanh
            u2_psum = psump.tile([1, H], F32, tag="U2", bufs=1)
            nc.vector.scalar_tensor_tensor(
                out=u2_psum,
                in0=u_psum,
                scalar=r_all[0:1, :],
                in1=q_inv,
                op0=mybir.AluOpType.mult,
                op1=mybir.AluOpType.mult,
            )
            out_sb = smallp.tile([1, H], F32, tag="out_sb")
            nc.scalar.activation(
                out=out_sb, in_=u2_psum, func=mybir.ActivationFunctionType.Tanh
            )
            nc.sync.dma_start(out=out_ap[b], in_=out_sb)


def kernel(
    output: np.ndarray,
    context: np.ndarray,
    _trace: bool = False,
    _repeat: int = 1,
):
    global LAST_RESULTS
    output = np.ascontiguousarray(np.asarray(output, dtype=np.float32))
    context = np.ascontiguousarray(np.asarray(context, dtype=np.float32))
    assert output.shape == (B, 1, H) and context.shape == (B, S, H)

    nc = bacc.Bacc(
        "TRN2",
        target_bir_lowering=False,
        debug=False,
        enable_asserts=False,
        num_devices=NCORES,
    )
    q_t = nc.dram_tensor("q", [BPC, 1, H], F32, kind="ExternalInput")
    c_t = nc.dram_tensor("c", [BPC, S, H], F32, kind="ExternalInput")
    out_t = nc.dram_tensor("out", [BPC, 1, H], F32, kind="ExternalOutput")
    attn_t = nc.dram_tensor("attn", [BPC, 1, S], F32, kind="ExternalOutput")

    with tile.TileContext(nc) as tc:
        _build(tc, q_t.ap(), c_t.ap(), out_t.ap(), attn_t.ap(), repeat=_repeat)
    nc.compile()

    in_maps = [
        {
            "q": output[i * BPC : (i + 1) * BPC],
            "c": context[i * BPC : (i + 1) * BPC],
        }
        for i in range(NCORES)
    ]
    res = run_bass_kernel_spmd(
        nc, in_maps, core_ids=list(range(NCORES)), trace=_trace
    )
    LAST_RESULTS = res
    out = np.concatenate([r["out"] for r in res.results], axis=0)
    attn = np.concatenate([r["attn"] for r in res.results], axis=0)
    return out, attn


if __name__ == "__main__":
    rng = np.random.default_rng(0)
    q = rng.standard_normal((B, 1, H), dtype=np.float32)
    c = rng.standard_normal((B, S, H), dtype=np.float32)
    o, a = kernel(q, c)
    print(o.shape, a.shape, float(np.abs(o).max()), float(a.sum(axis=-1).mean()))


# revision 11
# speedup vs baseline: 1.2194x; 1.0435x over previous
# Trainium2 Bass kernel for single-query dot-product attention (decode step).
#
#   attn = softmax(q @ C^T)          q: (B, 1, H)  C: (B, S, H)
#   out  = tanh(attn @ C)
#   returns (out, attn)              B=32, S=4096, H=1024, fp32
#
# Sharding: batch-parallel, 4 batches per core across 8 NeuronCores.
#
# Per-core dataflow (per batch, single HBM pass over C):
#   - DMA C in [128, 2, 1024] chunks (partition = s mod 128, 1 MiB per dma)
#   - scores: DVE tensor_tensor_reduce computes prod_t = C_t * q_rep
#     (written as float32r for the later PE matmul) and accumulates
#     scores[:, t] = sum_h prod in the same pass
#   - softmax: DVE free-axis max, GPSIMD partition all-reduce (max),
#     ACT exp-with-accum (row sums), GPSIMD all-reduce (add), normalize
#   - out' = P^T @ prod on PE in fp32r (attn weights stationary, prod
#     streaming, PSUM accumulation over all 32 s-subtiles). Since
#     prod = C .* q, out' = out .* q; divide by q (DVE, 2-ULP approx
#     reciprocal) and tanh on ACT.
#   - attn transposed on PE so the HBM writeout is contiguous.
from contextlib import ExitStack

import numpy as np

import concourse.bass as bass
import concourse.bacc as bacc
import concourse.mybir as mybir
import concourse.bass_isa as bass_isa
import concourse.tile as tile
from concourse.bass_utils import run_bass_kernel_spmd
from concourse.masks import make_identity

B, S, H = 32, 4096, 1024
NCORES = 8
BPC = B // NCORES          # batches per core
NT = S // 128              # 32 s-subtiles of 128 rows per batch
TPD = 4                    # s-subtiles per DMA chunk (2 MiB per dma_start)
ND = NT // TPD             # dma chunks per batch

F32 = mybir.dt.float32
F32R = mybir.dt.float32r

CPOOL_BUFS = 3
PROD_BUFS = 48
WARM_EVERY = 2             # dummy PE matmul every N score tiles (keeps HAM hot)
PROD_DT = mybir.dt.bfloat16  # fp32r for higher precision, bf16 for SBUF depth

LAST_RESULTS = None        # test.py reads profiling info from here


def _build(tc, q_ap, c_ap, out_ap, attn_ap, repeat=1):
    nc = tc.nc
    ctx = ExitStack()
    with ctx:
        cpool = ctx.enter_context(tc.tile_pool(name="cpool", bufs=CPOOL_BUFS))
        prodp = ctx.enter_context(tc.tile_pool(name="prodp", bufs=PROD_BUFS))
        qpool = ctx.enter_context(tc.tile_pool(name="qpool", bufs=2))
        smallp = ctx.enter_context(tc.tile_pool(name="smallp", bufs=2))
        singles = ctx.enter_context(tc.tile_pool(name="singles", bufs=1))
        psump = ctx.enter_context(tc.tile_pool(name="psump", bufs=2, space="PSUM"))

        identity = singles.tile([128, 128], F32)
        make_identity(nc, identity)
        warm_psum = psump.tile([1, 1], F32, tag="warm", bufs=1)

        if repeat > 1:
            # timing amplification only: run the whole per-core program
            # `repeat` times inside a device-side loop
            loop = ctx.enter_context(tc.For_i(0, repeat, 1))

        for b in range(BPC):
            # q broadcast to all 128 partitions: [128, H]
            q_rep = qpool.tile([128, H], F32)
            q_src = bass.AP(
                tensor=q_ap.tensor, offset=b * H, ap=[[0, 128], [1, H]]
            )
            nc.gpsimd.dma_start(out=q_rep, in_=q_src)

            # 1/q for the final un-scaling (prod tiles carry a factor of q)
            q_inv = smallp.tile([1, H], F32, tag="q_inv")
            q_inv_scratch = smallp.tile([1, H], F32, tag="q_inv_scratch")
            nc.vector.reciprocal_approx_accurate(
                out=q_inv, in_=q_rep[0:1, :], scratch=q_inv_scratch
            )

            # C for this batch: s = t*128 + p  ->  [p, t, h]
            c_resh = c_ap[b].rearrange("(t p) h -> p t h", p=128)

            scores = smallp.tile([128, NT], F32, tag="scores")
            prods = []
            for j in range(ND):
                c_tile = cpool.tile([128, TPD, H], F32, tag="C")
                nc.sync.dma_start(
                    out=c_tile, in_=c_resh[:, j * TPD : (j + 1) * TPD, :]
                )
                for k in range(TPD):
                    t = j * TPD + k
                    prod = prodp.tile([128, H], PROD_DT, tag="prod")
                    # prod = C_t * q (rounded to fp32r for the PE matmul),
                    # scores[:, t] = sum_h prod  — one DVE pass
                    nc.vector.scalar_tensor_tensor(
                        out=prod,
                        in0=c_tile[:, k, :],
                        scalar=0.0,
                        in1=q_rep,
                        op0=mybir.AluOpType.bypass,
                        op1=mybir.AluOpType.mult,
                        accum_out=scores[:, t : t + 1],
                    )
                    prods.append(prod)
                    if t % WARM_EVERY == 1:
                        # tiny dependent matmul: keeps the PE HAM window
                        # busy through the scores phase so the real burst
                        # runs at 2.4 GHz
                        nc.tensor.matmul(
                            warm_psum,
                            lhsT=scores[:, t : t + 1],
                            rhs=scores[:, t : t + 1],
                            start=True,
                            stop=True,
                        )

            # global max over the whole [128, NT] score block
            mx = smallp.tile([128, 1], F32, tag="mx")
            nc.vector.reduce_max(mx, scores, axis=mybir.AxisListType.X)
            m_all = smallp.tile([128, 1], F32, tag="m_all")
            nc.gpsimd.partition_all_reduce(
                m_all, mx, channels=128, reduce_op=bass_isa.ReduceOp.max
            )
            negm = smallp.tile([128, 1], F32, tag="negm")
            nc.vector.tensor_scalar_mul(negm, m_all, -1.0)

            # E = exp(scores - max), z_col[p] = sum_t E[p, t]
            e_blk = smallp.tile([128, NT], F32, tag="e_blk")
            z_col = smallp.tile([128, 1], F32, tag="z_col")
            nc.scalar.activation(
                out=e_blk,
                in_=scores,
                func=mybir.ActivationFunctionType.Exp,
                bias=negm,
                scale=1.0,
                accum_out=z_col,
            )
            # fp32r copy of E: the output matmul runs on UNNORMALIZED
            # weights so it can start right after the exp; 1/Z is folded
            # into the final [1, H] scale instead.
            e_r = smallp.tile([128, NT], PROD_DT, tag="e_r")
            nc.vector.tensor_scalar_mul(e_r, e_blk, 1.0)

            # out'' = E^T @ prod accumulated over all 32 s-subtiles, fp32r.
            # t outer so prod tiles free in order for the next batch.
            u_psum = psump.tile([1, H], F32, tag="U", bufs=1)
            for t in range(NT):
                for n in range(H // 512):
                    nc.tensor.matmul(
                        u_psum[:, n * 512 : (n + 1) * 512],
                        lhsT=e_r[:, t : t + 1],
                        rhs=prods[t][:, n * 512 : (n + 1) * 512],
                        start=(t == 0),
                        stop=(t == NT - 1),
                    )

            # Z and attention normalization (off the PE critical path)
            z_all = smallp.tile([128, 1], F32, tag="z_all")
            nc.gpsimd.partition_all_reduce(
                z_all, z_col, channels=128, reduce_op=bass_isa.ReduceOp.add
            )
            r_all = smallp.tile([128, 1], F32, tag="r_all")
            nc.vector.reciprocal(r_all, z_all)
            p_attn = smallp.tile([128, NT], F32, tag="p_attn")
            nc.vector.tensor_scalar_mul(p_attn, e_blk, r_all)

            # attn writeout: transpose [128, NT] -> [NT, 128] so HBM rows are
            # contiguous per partition
            pt_psum = psump.tile([NT, 128], F32, tag="pt")
            nc.tensor.transpose(pt_psum, p_attn, identity)
            attn_sb = smallp.tile([NT, 128], F32, tag="attn_sb")
            nc.scalar.copy(attn_sb, pt_psum)
            nc.sync.dma_start(
                out=attn_ap[b].rearrange("q (t j) -> (q t) j", j=128), in_=attn_sb
            )

            # out = tanh(out'' * (1/Z) / q) — single DVE pass + ACT tanh
            u2_psum = psump.tile([1, H], F32, tag="U2", bufs=1)
            nc.vector.scalar_tensor_tensor(
                out=u2_psum,
                in0=u_psum,
                scalar=r_all[0:1, :],
                in1=q_inv,
                op0=mybir.AluOpType.mult,
                op1=mybir.AluOpType.mult,
            )
            out_sb = smallp.tile([1, H], F32, tag="out_sb")
            nc.scalar.activation(
                out=out_sb, in_=u2_psum, func=mybir.ActivationFunctionType.Tanh
            )
            nc.sync.dma_start(out=out_ap[b], in_=out_sb)


def kernel(
    output: np.ndarray,
    context: np.ndarray,
    _trace: bool = False,
    _repeat: int = 1,
):
    global LAST_RESULTS
    output = np.ascontiguousarray(np.asarray(output, dtype=np.float32))
    context = np.ascontiguousarray(np.asarray(context, dtype=np.float32))
    assert output.shape == (B, 1, H) and context.shape == (B, S, H)

    nc = bacc.Bacc(
        "TRN2",
        target_bir_lowering=False,
        debug=False,
        enable_asserts=False,
        num_devices=NCORES,
    )
    q_t = nc.dram_tensor("q", [BPC, 1, H], F32, kind="ExternalInput")
    c_t = nc.dram_tensor("c", [BPC, S, H], F32, kind="ExternalInput")
    out_t = nc.dram_tensor("out", [BPC, 1, H], F32, kind="ExternalOutput")
    attn_t = nc.dram_tensor("attn", [BPC, 1, S], F32, kind="ExternalOutput")

    with tile.TileContext(nc) as tc:
        _build(tc, q_t.ap(), c_t.ap(), out_t.ap(), attn_t.ap(), repeat=_repeat)
    nc.compile()

    in_maps = [
        {
            "q": output[i * BPC : (i + 1) * BPC],
            "c": context[i * BPC : (i + 1) * BPC],
        }
        for i in range(NCORES)
    ]
    res = run_bass_kernel_spmd(
        nc, in_maps, core_ids=list(range(NCORES)), trace=_trace
    )
    LAST_RESULTS = res
    out = np.concatenate([r["out"] for r in res.results], axis=0)
    attn = np.concatenate([r["attn"] for r in res.results], axis=0)
    return out, attn


if __name__ == "__main__":
    rng = np.random.default_rng(0)
    q = rng.standard_normal((B, 1, H), dtype=np.float32)
    c = rng.standard_normal((B, S, H), dtype=np.float32)
    o, a = kernel(q, c)
    print(o.shape, a.shape, float(np.abs(o).max()), float(a.sum(axis=-1).mean()))


# revision 15
# speedup vs baseline: 1.2231x; 1.0030x over previous
# Trainium2 Bass kernel for single-query dot-product attention (decode step).
#
#   attn = softmax(q @ C^T)          q: (B, 1, H)  C: (B, S, H)
#   out  = tanh(attn @ C)
#   returns (out, attn)              B=32, S=4096, H=1024, fp32
#
# Sharding: batch-parallel, 4 batches per core across 8 NeuronCores.
#
# Per-core dataflow (per batch, single HBM pass over C):
#   - DMA C in [128, 2, 1024] chunks (partition = s mod 128, 1 MiB per dma)
#   - scores: DVE tensor_tensor_reduce computes prod_t = C_t * q_rep
#     (written as float32r for the later PE matmul) and accumulates
#     scores[:, t] = sum_h prod in the same pass
#   - softmax: DVE free-axis max, GPSIMD partition all-reduce (max),
#     ACT exp-with-accum (row sums), GPSIMD all-reduce (add), normalize
#   - out' = P^T @ prod on PE in fp32r (attn weights stationary, prod
#     streaming, PSUM accumulation over all 32 s-subtiles). Since
#     prod = C .* q, out' = out .* q; divide by q (DVE, 2-ULP approx
#     reciprocal) and tanh on ACT.
#   - attn transposed on PE so the HBM writeout is contiguous.
from contextlib import ExitStack

import numpy as np

import concourse.bass as bass
import concourse.bacc as bacc
import concourse.mybir as mybir
import concourse.bass_isa as bass_isa
import concourse.tile as tile
from concourse.bass_utils import run_bass_kernel_spmd
from concourse.masks import make_identity

B, S, H = 32, 4096, 1024
NCORES = 8
BPC = B // NCORES          # batches per core
NT = S // 128              # 32 s-subtiles of 128 rows per batch
TPD = 4                    # s-subtiles per DMA chunk (2 MiB per dma_start)
ND = NT // TPD             # dma chunks per batch

F32 = mybir.dt.float32
F32R = mybir.dt.float32r

CPOOL_BUFS = 3
PROD_BUFS = 56
WARM_EVERY = 2             # dummy PE matmul every N score tiles (keeps HAM hot)
PROD_DT = mybir.dt.bfloat16  # fp32r for higher precision, bf16 for SBUF depth

LAST_RESULTS = None        # test.py reads profiling info from here


def _build(tc, q_ap, c_ap, out_ap, attn_ap, repeat=1):
    nc = tc.nc
    ctx = ExitStack()
    with ctx:
        cpool = ctx.enter_context(tc.tile_pool(name="cpool", bufs=CPOOL_BUFS))
        prodp = ctx.enter_context(tc.tile_pool(name="prodp", bufs=PROD_BUFS))
        qpool = ctx.enter_context(tc.tile_pool(name="qpool", bufs=2))
        smallp = ctx.enter_context(tc.tile_pool(name="smallp", bufs=2))
        singles = ctx.enter_context(tc.tile_pool(name="singles", bufs=1))
        psump = ctx.enter_context(tc.tile_pool(name="psump", bufs=2, space="PSUM"))

        identity = singles.tile([128, 128], F32)
        make_identity(nc, identity)
        warm_psum = psump.tile([1, 1], F32, tag="warm", bufs=1)

        if repeat > 1:
            # timing amplification only: run the whole per-core program
            # `repeat` times inside a device-side loop
            loop = ctx.enter_context(tc.For_i(0, repeat, 1))

        TPH = NT // 2          # score tiles per half
        CPH = ND // 2          # dma chunks per half

        for b in range(BPC):
            # q broadcast to all 128 partitions: [128, H]
            q_rep = qpool.tile([128, H], F32)
            q_src = bass.AP(
                tensor=q_ap.tensor, offset=b * H, ap=[[0, 128], [1, H]]
            )
            nc.gpsimd.dma_start(out=q_rep, in_=q_src)

            # 1/q for the final un-scaling (prod tiles carry a factor of q)
            q_inv = smallp.tile([1, H], F32, tag="q_inv")
            q_inv_scratch = smallp.tile([1, H], F32, tag="vec_scratch")
            nc.vector.reciprocal_approx_accurate(
                out=q_inv, in_=q_rep[0:1, :], scratch=q_inv_scratch
            )

            # C for this batch: s = t*128 + p  ->  [p, t, h]
            c_resh = c_ap[b].rearrange("(t p) h -> p t h", p=128)

            scores = smallp.tile([128, NT], F32, tag="scores")
            e_blk = smallp.tile([128, NT], F32, tag="e_blk")
            e_r = smallp.tile([128, NT], PROD_DT, tag="e_r")
            # per-half running stats: col 0 = half A, col 1 = half B
            mAB = smallp.tile([128, 2], F32, tag="mAB")
            zAB = smallp.tile([128, 2], F32, tag="zAB")
            u_ps = [
                psump.tile([1, H], F32, tag="UA", bufs=1, name="u_ps_a"),
                psump.tile([1, H], F32, tag="UB", bufs=1, name="u_ps_b"),
            ]
            prods = []

            # Each half: stream chunks + scores, then a local softmax with
            # the half's own max, then that half's PE accumulation — so the
            # first half's matmuls overlap the second half's DMA.
            for hlf in range(2):
                t0 = hlf * TPH
                for j in range(hlf * CPH, (hlf + 1) * CPH):
                    c_tile = cpool.tile([128, TPD, H], F32, tag="C")
                    nc.sync.dma_start(
                        out=c_tile, in_=c_resh[:, j * TPD : (j + 1) * TPD, :]
                    )
                    for k in range(TPD):
                        t = j * TPD + k
                        prod = prodp.tile([128, H], PROD_DT, tag="prod")
                        # prod = C_t * q (bf16 for the PE matmul),
                        # scores[:, t] = sum_h C_t*q  — one DVE pass
                        nc.vector.scalar_tensor_tensor(
                            out=prod,
                            in0=c_tile[:, k, :],
                            scalar=0.0,
                            in1=q_rep,
                            op0=mybir.AluOpType.bypass,
                            op1=mybir.AluOpType.mult,
                            accum_out=scores[:, t : t + 1],
                        )
                        prods.append(prod)
                        if t % WARM_EVERY == 1:
                            # tiny dependent matmul: keeps the PE HAM window
                            # busy through the scores phase so the real
                            # bursts run at 2.4 GHz
                            nc.tensor.matmul(
                                warm_psum,
                                lhsT=scores[:, t : t + 1],
                                rhs=scores[:, t : t + 1],
                                start=True,
                                stop=True,
                            )

                # local max over this half's score columns
                mx = smallp.tile([128, 1], F32, tag="mx", bufs=4)
                nc.vector.reduce_max(
                    mx, scores[:, t0 : t0 + TPH], axis=mybir.AxisListType.X
                )
                nc.gpsimd.partition_all_reduce(
                    mAB[:, hlf : hlf + 1], mx, channels=128,
                    reduce_op=bass_isa.ReduceOp.max,
                )
                negm = smallp.tile([128, 1], F32, tag="negm", bufs=4)
                nc.vector.tensor_scalar_mul(negm, mAB[:, hlf : hlf + 1], -1.0)

                # E_half = exp(scores_half - m_half), z accumulated per row
                nc.scalar.activation(
                    out=e_blk[:, t0 : t0 + TPH],
                    in_=scores[:, t0 : t0 + TPH],
                    func=mybir.ActivationFunctionType.Exp,
                    bias=negm,
                    scale=1.0,
                    accum_out=zAB[:, hlf : hlf + 1],
                )
                nc.vector.tensor_scalar_mul(
                    e_r[:, t0 : t0 + TPH], e_blk[:, t0 : t0 + TPH], 1.0
                )

                # this half's PE accumulation (unnormalized, local max)
                for t in range(t0, t0 + TPH):
                    for n in range(H // 512):
                        nc.tensor.matmul(
                            u_ps[hlf][:, n * 512 : (n + 1) * 512],
                            lhsT=e_r[:, t : t + 1],
                            rhs=prods[t][:, n * 512 : (n + 1) * 512],
                            start=(t == t0),
                            stop=(t == t0 + TPH - 1),
                        )

            # merge the two halves: alpha_h = exp(m_h - m), with
            # m = max(m_A, m_B)
            m_all = smallp.tile([128, 1], F32, tag="m_all")
            nc.vector.reduce_max(m_all, mAB, axis=mybir.AxisListType.X)
            negm_g = smallp.tile([128, 1], F32, tag="negm_g")
            nc.vector.tensor_scalar_mul(negm_g, m_all, -1.0)
            alphas = smallp.tile([128, 2], F32, tag="alphas")
            nc.scalar.activation(
                out=alphas, in_=mAB,
                func=mybir.ActivationFunctionType.Exp,
                bias=negm_g, scale=1.0,
            )
            # Z = alpha_A * z_A + alpha_B * z_B, reduced over partitions
            zb_s = smallp.tile([128, 1], F32, tag="zb_s")
            nc.vector.tensor_scalar_mul(zb_s, zAB[:, 1:2], alphas[:, 1:2])
            z_comb = smallp.tile([128, 1], F32, tag="z_comb")
            nc.vector.scalar_tensor_tensor(
                out=z_comb, in0=zAB[:, 0:1], scalar=alphas[:, 0:1],
                in1=zb_s, op0=mybir.AluOpType.mult, op1=mybir.AluOpType.add,
            )
            z_all = smallp.tile([128, 1], F32, tag="z_all")
            nc.gpsimd.partition_all_reduce(
                z_all, z_comb, channels=128, reduce_op=bass_isa.ReduceOp.add
            )
            r_all = smallp.tile([128, 1], F32, tag="r_all")
            nc.vector.reciprocal(r_all, z_all)
            # per-half combined scale alpha_h / Z
            sAB = smallp.tile([128, 2], F32, tag="sAB")
            nc.vector.tensor_scalar_mul(sAB, alphas, r_all)

            # normalized attention weights
            p_attn = smallp.tile([128, NT], F32, tag="p_attn")
            nc.vector.tensor_scalar_mul(
                p_attn[:, 0:TPH], e_blk[:, 0:TPH], sAB[:, 0:1]
            )
            nc.vector.tensor_scalar_mul(
                p_attn[:, TPH:NT], e_blk[:, TPH:NT], sAB[:, 1:2]
            )

            # attn writeout: transpose [128, NT] -> [NT, 128] so HBM rows are
            # contiguous per partition
            pt_psum = psump.tile([NT, 128], F32, tag="pt")
            nc.tensor.transpose(pt_psum, p_attn, identity)
            attn_sb = smallp.tile([NT, 128], F32, tag="attn_sb")
            nc.scalar.copy(attn_sb, pt_psum)
            nc.sync.dma_start(
                out=attn_ap[b].rearrange("q (t j) -> (q t) j", j=128), in_=attn_sb
            )

            # out = tanh((s_A*U_A + s_B*U_B) / q)
            ub_sb = smallp.tile([1, H], F32, tag="vec_scratch")
            nc.scalar.activation(
                out=ub_sb, in_=u_ps[1],
                func=mybir.ActivationFunctionType.Copy,
                scale=sAB[0:1, 1:2],
            )
            u_sb = smallp.tile([1, H], F32, tag="u_sb")
            nc.vector.scalar_tensor_tensor(
                out=u_sb, in0=u_ps[0], scalar=sAB[0:1, 0:1], in1=ub_sb,
                op0=mybir.AluOpType.mult, op1=mybir.AluOpType.add,
            )
            nc.vector.tensor_mul(u_sb, u_sb, q_inv)
            out_sb = smallp.tile([1, H], F32, tag="out_sb")
            nc.scalar.activation(
                out=out_sb, in_=u_sb, func=mybir.ActivationFunctionType.Tanh
            )
            nc.sync.dma_start(out=out_ap[b], in_=out_sb)


def kernel(
    output: np.ndarray,
    context: np.ndarray,
    _trace: bool = False,
    _repeat: int = 1,
):
    global LAST_RESULTS
    output = np.ascontiguousarray(np.asarray(output, dtype=np.float32))
    context = np.ascontiguousarray(np.asarray(context, dtype=np.float32))
    assert output.shape == (B, 1, H) and context.shape == (B, S, H)

    nc = bacc.Bacc(
        "TRN2",
        target_bir_lowering=False,
        debug=False,
        enable_asserts=False,
        num_devices=NCORES,
    )
    q_t = nc.dram_tensor("q", [BPC, 1, H], F32, kind="ExternalInput")
    c_t = nc.dram_tensor("c", [BPC, S, H], F32, kind="ExternalInput")
    out_t = nc.dram_tensor("out", [BPC, 1, H], F32, kind="ExternalOutput")
    attn_t = nc.dram_tensor("attn", [BPC, 1, S], F32, kind="ExternalOutput")

    with tile.TileContext(nc) as tc:
        _build(tc, q_t.ap(), c_t.ap(), out_t.ap(), attn_t.ap(), repeat=_repeat)
    nc.compile()

    in_maps = [
        {
            "q": output[i * BPC : (i + 1) * BPC],
            "c": context[i * BPC : (i + 1) * BPC],
        }
        for i in range(NCORES)
    ]
    res = run_bass_kernel_spmd(
        nc, in_maps, core_ids=list(range(NCORES)), trace=_trace
    )
    LAST_RESULTS = res
    out = np.concatenate([r["out"] for r in res.results], axis=0)
    attn = np.concatenate([r["attn"] for r in res.results], axis=0)
    return out, attn


if __name__ == "__main__":
    rng = np.random.default_rng(0)
    q = rng.standard_normal((B, 1, H), dtype=np.float32)
    c = rng.standard_normal((B, S, H), dtype=np.float32)
    o, a = kernel(q, c)
    print(o.shape, a.shape, float(np.abs(o).max()), float(a.sum(axis=-1).mean()))
